# revision 8
# baseline (speedup 1.0000x reference)
"""Trainium2 Bass kernel for nn_BatchedLSTM (B=16, F=2048, C=512, H=512).

Strategy:
  - Shard batch dim B=16 over 8 NeuronCores (2 batches/core); replicate the
    fused gate weight matrix on every core.
  - Host-side prep: transpose x to (B, C, F) so the contraction dim (C+H)
    lands on SBUF partitions with fully-contiguous DMA; pre-concatenate and
    pre-transpose the 4 gate weights to W_T (C+H, 4H).
  - Device: gates = W_T.T @ [x^T; h] computed as 128x128-stationary fp32r
    matmuls with N=512 moving tiles accumulated over 8 K-tiles in PSUM,
    bias+sigmoid/tanh fused on the scalar engine straight out of PSUM,
    LSTM cell update on the vector engine.
  - Outputs new_hidden/new_cell in natural (B, H, F) layout; the (B, F, H)
    output is produced host-side as a transposed view (zero device cost).
"""

import sys

sys.path.insert(0, "/opt/trn_rl_repo")

import numpy as np

B, F, C, H = 16, 2048, 512, 512
NCORES = 8
BPC = B // NCORES          # batches per core
K = C + H                  # contraction dim
M4H = 4 * H                # fused gate output channels
P = 128                    # SBUF partitions
FT = 512                   # moving-tile frames (max fp32 moving free dim)
NKT = K // P               # 8 k-tiles
NXT = C // P               # 4 k-tiles from x
NFT = F // FT              # 4 frame chunks per batch
NHT = H // P               # 4 h-tiles (gate partition tiles per gate)

_CACHE = {}


def _build_nc():
    import concourse.bass as bass
    import concourse.mybir as mybir
    import concourse.tile as tile
    from concourse import bacc

    f32 = mybir.dt.float32
    f32r = mybir.dt.float32r
    AF = mybir.ActivationFunctionType

    nc = bacc.Bacc(None, target_bir_lowering=False)

    # x / hidden / W feed the fp32r matmul (host pre-rounds them to the
    # fp32r grid); cell and bias stay full fp32 (elementwise-only).
    xt = nc.dram_tensor("xt", [BPC, C, F], f32r, kind="ExternalInput")
    hid = nc.dram_tensor("hid", [BPC, H, F], f32r, kind="ExternalInput")
    cel = nc.dram_tensor("cel", [BPC, H, F], f32, kind="ExternalInput")
    wt = nc.dram_tensor("wt", [K, M4H], f32r, kind="ExternalInput")
    bias = nc.dram_tensor("bias", [M4H], f32, kind="ExternalInput")
    nh = nc.dram_tensor("nh", [BPC, H, F], f32, kind="ExternalOutput")
    ncl = nc.dram_tensor("ncl", [BPC, H, F], f32, kind="ExternalOutput")

    # k = kt*128 + p; m = channel of the fused 4H gate dim
    wt_r = wt[:].rearrange("(kt p) m -> p kt m", p=P)          # (128, 8, 2048)
    bias_r = bias[:].rearrange("(mt p) -> p mt", p=P)          # (128, 16)
    xt_r = xt[:].rearrange("b (kt p) f -> b p kt f", p=P)      # (2, 128, 4, F)
    hid_r = hid[:].rearrange("b (kt p) f -> b p kt f", p=P)    # (2, 128, 4, F)
    cel_r = cel[:].rearrange("b (ht p) f -> b p ht f", p=P)    # (2, 128, 4, F)
    nh_r = nh[:].rearrange("b (ht p) f -> b p ht f", p=P)
    ncl_r = ncl[:].rearrange("b (ht p) f -> b p ht f", p=P)

    # gate order in the fused weight: f, i, g, o (matches reference concat)
    gate_funcs = [AF.Sigmoid, AF.Sigmoid, AF.Tanh, AF.Sigmoid]

    with tile.TileContext(nc) as tc:
        with (
            tc.tile_pool(name="wpool", bufs=1) as wpool,
            tc.tile_pool(name="xpool", bufs=2) as xpool,
            tc.tile_pool(name="cpool", bufs=2) as cpool,
            tc.tile_pool(name="gpool", bufs=6) as gpool,
            tc.tile_pool(name="tpool", bufs=3) as tpool,
            tc.tile_pool(name="opool", bufs=3) as opool,
            tc.tile_pool(name="pspool", bufs=6, space="PSUM") as pspool,
        ):
            w_sb = wpool.tile([P, NKT, M4H], f32r)
            nc.sync.dma_start(out=w_sb[:], in_=wt_r)
            b_sb = wpool.tile([P, M4H // P], f32)
            nc.sync.dma_start(out=b_sb[:], in_=bias_r)

            for b in range(BPC):
                for nf in range(NFT):
                    fsl = slice(nf * FT, (nf + 1) * FT)
                    xh = xpool.tile([P, NKT, FT], f32r)
                    nc.sync.dma_start(out=xh[:, 0:NXT, :], in_=xt_r[b, :, :, fsl])
                    nc.sync.dma_start(out=xh[:, NXT:NKT, :], in_=hid_r[b, :, :, fsl])
                    cell_sb = cpool.tile([P, NHT, FT], f32)
                    nc.sync.dma_start(out=cell_sb[:], in_=cel_r[b, :, :, fsl])

                    for hi in range(NHT):
                        gts = []
                        for j in range(4):
                            mi = j * NHT + hi
                            ps = pspool.tile([P, FT], f32)
                            for kt in range(NKT):
                                nc.tensor.matmul(
                                    ps[:],
                                    lhsT=w_sb[:, kt, mi * P:(mi + 1) * P],
                                    rhs=xh[:, kt, :],
                                    start=(kt == 0),
                                    stop=(kt == NKT - 1),
                                )
                            gt = gpool.tile([P, FT], f32)
                            nc.scalar.activation(
                                gt[:], ps[:], gate_funcs[j],
                                bias=b_sb[:, mi:mi + 1], scale=1.0,
                            )
                            gts.append(gt)
                        fg, ig, gg, og = gts

                        t1 = tpool.tile([P, FT], f32)
                        nc.vector.tensor_mul(out=t1[:], in0=ig[:], in1=gg[:])
                        t2 = tpool.tile([P, FT], f32)
                        nc.vector.tensor_mul(out=t2[:], in0=cell_sb[:, hi, :], in1=fg[:])
                        ncell = opool.tile([P, FT], f32)
                        nc.vector.tensor_add(out=ncell[:], in0=t1[:], in1=t2[:])
                        nc.sync.dma_start(out=ncl_r[b, :, hi, fsl], in_=ncell[:])

                        th = tpool.tile([P, FT], f32)
                        nc.scalar.activation(th[:], ncell[:], AF.Tanh)
                        nht = opool.tile([P, FT], f32)
                        nc.vector.tensor_mul(out=nht[:], in0=th[:], in1=og[:])
                        nc.sync.dma_start(out=nh_r[b, :, hi, fsl], in_=nht[:])

    nc.compile()
    return nc


def _get_nc():
    if "nc" not in _CACHE:
        _CACHE["nc"] = _build_nc()
    return _CACHE["nc"]


def _round_fp32r(a):
    """Round fp32 values to the fp32r grid (RNE, low 12 mantissa bits zeroed),
    matching neuronxcc's static_cast(..., float32r)."""
    b = np.ascontiguousarray(a, np.float32).view(np.uint32)
    low = b & np.uint32(0xFFF)
    up = (low > 0x800) | ((low == 0x800) & (((b >> np.uint32(12)) & np.uint32(1)) == 1))
    b = (b & np.uint32(0xFFFFF000)) + np.where(up, np.uint32(0x1000), np.uint32(0))
    return b.view(np.float32)


def kernel(x, hidden_state, cell_state, W_f, b_f, W_i, b_i, W_o, b_o, W_g, b_g):
    from concourse.bass_utils import run_bass_kernel_spmd

    nc = _get_nc()

    x_t = _round_fp32r(np.swapaxes(np.asarray(x, np.float32), 1, 2))
    hidden_state = _round_fp32r(np.asarray(hidden_state, np.float32))
    cell_state = np.ascontiguousarray(np.asarray(cell_state, np.float32))
    W = np.concatenate([W_f, W_i, W_g, W_o], axis=0).astype(np.float32)
    wt = _round_fp32r(W.T)                              # (K, 4H)
    bias = np.concatenate([b_f, b_i, b_g, b_o]).astype(np.float32)

    in_maps = []
    for c in range(NCORES):
        sl = slice(c * BPC, (c + 1) * BPC)
        in_maps.append({
            "xt": np.ascontiguousarray(x_t[sl]),
            "hid": np.ascontiguousarray(hidden_state[sl]),
            "cel": np.ascontiguousarray(cell_state[sl]),
            "wt": wt,
            "bias": bias,
        })

    res = run_bass_kernel_spmd(nc, in_maps, list(range(NCORES)))
    new_hidden = np.concatenate([r["nh"] for r in res.results], axis=0)
    new_cell = np.concatenate([r["ncl"] for r in res.results], axis=0)
    return (np.swapaxes(new_hidden, 1, 2), new_hidden, new_cell)
